# revision 3
# baseline (speedup 1.0000x reference)
"""NeighborAttention (B=4, N=4096, K=32, C=128, H=4) on 8 Trainium2 cores.

v2 design (engine-balanced, bf16 PE path, j-major chunks):

  Per core: 2048 nodes in 4 chunks of CN=512, ET stored j-major per chunk
  ([c, chunk, j, n]) in bf16. Per chunk:
    1a  PE: q = Wq'@x; per j: kt_j = Wk'@et_j (bf16);  DVE: prod_j = kt_j*q
    1b  PE: per j: s_j = Hrep@prod_j;  ACT: e_j = exp(s_j) (bf16)
    z   GpSimd: z = tree-sum_j e_j  (bf16 adds, f32-ish finish)
    2a  PE: per j: vt_j = Wv'@et_j;   DVE: uv_j = e_j*vt_j (bf16)
    2b  PE: usum = sum_j uv_j via 32 accumulating identity matmuls (f32 PSUM)
    max DVE: umax = tree-max_j uv_j (bf16 TT-MAX at 2x, offset-aligned)
    epi GpSimd: zc = max(z - mcorr, eps); ACT: rz = exp(-ln zc);
        DVE: wsn = usum*rz, mxn = umax*rz;  PE: out = Wos'@wsn + Wo3'@mxn

  Stationaries are phase-batched (one LDWEIGHTS per 32 matmuls).
  attn sums to exactly 1 so the mean/sum W_O blocks fold on the host;
  masked neighbors are zeroed on the host and contribute exp(0)=1 to z,
  corrected by the host-sent (K - count) term.
"""
import numpy as np
import ml_dtypes
import concourse.bass as bass
import concourse.bacc as bacc
import concourse.mybir as mybir
from concourse import tile
from concourse.bass_utils import run_bass_kernel_spmd

F32 = mybir.dt.float32
BF16 = mybir.dt.bfloat16
ALU = mybir.AluOpType
EXP = mybir.ActivationFunctionType.Exp
LN = mybir.ActivationFunctionType.Ln

K = 32
C = 128
H = 4
D = 32
NCORES = 8
CN = 512              # nodes per chunk
CC = K * CN           # et columns per chunk (16384)

_NC_CACHE = {}


def build_nc(nloc):
    if nloc in _NC_CACHE:
        return _NC_CACHE[nloc]
    nchunks = nloc // CN
    assert nloc % CN == 0

    nc = bacc.Bacc()
    et_d = nc.dram_tensor("et", [C, nloc * K], BF16, kind="ExternalInput")
    xt_d = nc.dram_tensor("xt", [C, nloc], BF16, kind="ExternalInput")
    wq_d = nc.dram_tensor("wq", [C, C], BF16, kind="ExternalInput")
    wk_d = nc.dram_tensor("wk", [C, C], BF16, kind="ExternalInput")
    wv_d = nc.dram_tensor("wv", [C, C], BF16, kind="ExternalInput")
    hr_d = nc.dram_tensor("hr", [C, C], BF16, kind="ExternalInput")
    id_d = nc.dram_tensor("idn", [C, C], BF16, kind="ExternalInput")
    wos_d = nc.dram_tensor("wos", [C, C], BF16, kind="ExternalInput")
    wo3_d = nc.dram_tensor("wo3", [C, C], BF16, kind="ExternalInput")
    mc_d = nc.dram_tensor("mc", [C, nloc], BF16, kind="ExternalInput")
    out_d = nc.dram_tensor("out", [C, nloc], F32, kind="ExternalOutput")

    with tile.TileContext(nc) as tc:
        with tc.tile_pool(name="wts", bufs=1) as wpool, \
             tc.tile_pool(name="glob", bufs=1) as gpool, \
             tc.tile_pool(name="etp", bufs=2) as etpool, \
             tc.tile_pool(name="big", bufs=1) as bigp, \
             tc.tile_pool(name="qp", bufs=2) as qpool, \
             tc.tile_pool(name="epi", bufs=1) as epip, \
             tc.tile_pool(name="pkv", bufs=2, space="PSUM") as pkv, \
             tc.tile_pool(name="psc", bufs=2, space="PSUM") as psc, \
             tc.tile_pool(name="psu", bufs=1, space="PSUM") as psu, \
             tc.tile_pool(name="pso", bufs=2, space="PSUM") as pso:

            w_q = wpool.tile([C, C], BF16, tag="wq")
            w_k = wpool.tile([C, C], BF16, tag="wk")
            w_v = wpool.tile([C, C], BF16, tag="wv")
            w_h = wpool.tile([C, C], BF16, tag="wh")
            w_i = wpool.tile([C, C], BF16, tag="wi")
            w_os = wpool.tile([C, C], BF16, tag="wos")
            w_o3 = wpool.tile([C, C], BF16, tag="wo3")
            for t, dd in ((w_q, wq_d), (w_k, wk_d), (w_v, wv_d), (w_h, hr_d),
                          (w_i, id_d), (w_os, wos_d), (w_o3, wo3_d)):
                nc.sync.dma_start(t[:], dd[:])

            xt_sb = gpool.tile([C, nloc], BF16, tag="xt")
            nc.sync.dma_start(xt_sb[:], xt_d[:])
            mc_sb = gpool.tile([C, nloc], BF16, tag="mc")
            nc.sync.dma_start(mc_sb[:], mc_d[:])

            umax_n = gpool.tile([C, 2 * CN], BF16, tag="umax")
            usum_n = gpool.tile([C, 2 * CN], BF16, tag="usum")
            z_n = gpool.tile([C, 2 * CN], BF16, tag="zn")

            prod_ch = bigp.tile([C, CC], BF16, tag="prod")
            e_ch = bigp.tile([C, CC], BF16, tag="ech")
            uv_ch = bigp.tile([C, CC], BF16, tag="uvch")
            zB = bigp.tile([C, CC // 2], BF16, tag="zB")

            def emit_epi_a(ech):
                en0 = ech * CN
                ep = (ech % 2) * CN
                zc = epip.tile([C, CN], F32, tag="zc")
                nc.gpsimd.tensor_sub(zc[:], z_n[:, ep:ep + CN],
                                     mc_sb[:, en0:en0 + CN])
                nc.gpsimd.tensor_scalar_max(zc[:], zc[:], 1e-20)
                nc.scalar.activation(zc[:], zc[:], LN)
                rz = epip.tile([C, CN], F32, tag="rz")
                nc.scalar.activation(rz[:], zc[:], EXP, scale=-1.0)
                wsn = epip.tile([C, CN], BF16, tag="wsn")
                nc.vector.tensor_mul(wsn[:], usum_n[:, ep:ep + CN], rz[:])
                return rz, wsn

            def emit_epi_b(ech, rz, wsn):
                en0 = ech * CN
                ep = (ech % 2) * CN
                mxn = epip.tile([C, CN], BF16, tag="mxn")
                nc.vector.tensor_mul(mxn[:], umax_n[:, ep:ep + CN], rz[:])
                o_ps = pso.tile([C, CN], F32, tag="qo")
                nc.tensor.matmul(o_ps[:], w_os[:], wsn[:], start=True, stop=False)
                nc.tensor.matmul(o_ps[:], w_o3[:], mxn[:], start=False, stop=True)
                o_sb = qpool.tile([C, CN], F32, tag="osb")
                nc.scalar.copy(o_sb[:], o_ps[:])
                nc.sync.dma_start(out_d[:, en0:en0 + CN], o_sb[:])

            def emit_epilogue(ech):
                rz, wsn = emit_epi_a(ech)
                emit_epi_b(ech, rz, wsn)

            HJ = K // 2  # 16 j per half-phase
            for ch in range(nchunks):
                c0 = ch * CC
                n0 = ch * CN
                et_sb = etpool.tile([C, CC], BF16, tag="et")
                nc.sync.dma_start(et_sb[:, :CC // 2], et_d[:, c0:c0 + CC // 2])
                nc.sync.dma_start(et_sb[:, CC // 2:], et_d[:, c0 + CC // 2:c0 + CC])

                q_ps = pso.tile([C, CN], F32, tag="qo")
                nc.tensor.matmul(q_ps[:], w_q[:], xt_sb[:, n0:n0 + CN],
                                 start=True, stop=True)
                q_f = qpool.tile([C, CN], F32, tag="qf")
                nc.scalar.copy(q_f[:], q_ps[:])

                # interleaved half-phases: 1a/1b per 16-j half, z-half after exp
                for hh in range(2):
                    j0 = hh * HJ
                    for j in range(j0, j0 + HJ):
                        kt = pkv.tile([C, CN], F32, tag="kv")
                        nc.tensor.matmul(kt[:], w_k[:],
                                         et_sb[:, j * CN:(j + 1) * CN],
                                         start=True, stop=True)
                        nc.vector.tensor_mul(prod_ch[:, j * CN:(j + 1) * CN],
                                             kt[:], q_f[:])
                    for j in range(j0, j0 + HJ):
                        s_ps = psc.tile([C, CN], F32, tag="s")
                        nc.tensor.matmul(s_ps[:], w_h[:],
                                         prod_ch[:, j * CN:(j + 1) * CN],
                                         start=True, stop=True)
                        nc.scalar.activation(e_ch[:, j * CN:(j + 1) * CN],
                                             s_ps[:], EXP)
                    # z half-sum: 16 slabs -> 8 slabs (4096 cols)
                    hb = hh * (CC // 4)
                    eb = hh * (CC // 2)
                    nc.gpsimd.tensor_add(zB[:, hb:hb + CC // 4],
                                         e_ch[:, eb:eb + CC // 4],
                                         e_ch[:, eb + CC // 4:eb + CC // 2])

                if ch > 0:
                    emit_epilogue(ch - 1)

                # 2a/2b interleaved halves
                u_ps = psu.tile([C, CN], F32, tag="u")
                for hh in range(2):
                    j0 = hh * HJ
                    for j in range(j0, j0 + HJ):
                        vt = pkv.tile([C, CN], F32, tag="kv")
                        nc.tensor.matmul(vt[:], w_v[:],
                                         et_sb[:, j * CN:(j + 1) * CN],
                                         start=True, stop=True)
                        nc.vector.tensor_mul(uv_ch[:, j * CN:(j + 1) * CN],
                                             e_ch[:, j * CN:(j + 1) * CN], vt[:])
                    for j in range(j0, j0 + HJ):
                        nc.tensor.matmul(u_ps[:], w_i[:],
                                         uv_ch[:, j * CN:(j + 1) * CN],
                                         start=(j == 0), stop=(j == K - 1))

                nc.scalar.copy(usum_n[:, (ch % 2) * CN:(ch % 2 + 1) * CN],
                               u_ps[:])

                # finish z tree (GpSimd): 16 slabs in zB -> z_n slab
                w = CC // 4  # 4096
                nc.gpsimd.tensor_add(zB[:, :w], zB[:, :w], zB[:, w:2 * w])
                w //= 2
                while w > CN:
                    nc.gpsimd.tensor_add(zB[:, :w], zB[:, :w], zB[:, w:2 * w])
                    w //= 2
                nc.gpsimd.tensor_add(z_n[:, (ch % 2) * CN:(ch % 2 + 1) * CN],
                                     zB[:, :CN], zB[:, CN:2 * CN])

                if ch == nchunks - 1:
                    _last_epi = emit_epi_a(ch)

                # max tree (DVE, offset-0 bf16 TT-MAX); s3 scratch aliases e_ch
                hf = CC // 2
                nc.vector.tensor_max(uv_ch[:, :hf], uv_ch[:, :hf], uv_ch[:, hf:])
                nc.vector.tensor_max(uv_ch[:, :hf // 2], uv_ch[:, :hf // 2],
                                     uv_ch[:, hf // 2:hf])
                w = hf // 4  # 2048
                while w >= CN:
                    nc.vector.tensor_copy(e_ch[:, :w], uv_ch[:, w:2 * w])
                    if w > CN:
                        nc.vector.tensor_max(uv_ch[:, :w], uv_ch[:, :w],
                                             e_ch[:, :w])
                    else:
                        nc.vector.tensor_max(
                            umax_n[:, (ch % 2) * CN:(ch % 2 + 1) * CN],
                            uv_ch[:, :w], e_ch[:, :w])
                    w //= 2


    nc.compile()
    _NC_CACHE[nloc] = nc
    return nc


def _perm_dh(w):
    """torch-layout [cout=(h*32+d), cin] -> lhsT [cin, cout2=(4d+h)]"""
    wt = np.asarray(w).reshape(H, D, -1)
    return np.ascontiguousarray(np.transpose(wt, (2, 1, 0)).reshape(-1, H * D))


def prep_inputs(h_X, h_E, mask_attn, W_Q, W_K, W_V, W_O):
    h_X = np.asarray(h_X, dtype=np.float32)
    h_E = np.asarray(h_E, dtype=np.float32)
    mask_attn = np.asarray(mask_attn)
    W_Q = np.asarray(W_Q, dtype=np.float32)
    W_K = np.asarray(W_K, dtype=np.float32)
    W_V = np.asarray(W_V, dtype=np.float32)
    W_O = np.asarray(W_O, dtype=np.float32)

    B, N, Kn, Cin = h_E.shape
    BN = B * N
    nloc = BN // NCORES
    nchunks = nloc // CN

    maskf = mask_attn.astype(np.float32)
    e_m = (h_E * maskf[..., None]).reshape(BN, Kn, Cin)
    xf = h_X.reshape(BN, -1)
    cnt = maskf.reshape(BN, Kn).sum(axis=1)

    bf = ml_dtypes.bfloat16
    wq = _perm_dh(W_Q / np.sqrt(D)).astype(bf)
    wk = _perm_dh(W_K).astype(bf)
    wv = _perm_dh(W_V).astype(bf)

    idx = np.arange(C)
    hh = idx % H
    hrep = (hh[:, None] == hh[None, :]).astype(bf)
    ident = np.eye(C, dtype=np.float32).astype(bf)

    wos = W_O[:, :C] + W_O[:, C:2 * C]
    wo3 = W_O[:, 2 * C:]
    wost = np.ascontiguousarray(
        wos.T.reshape(H, D, C).transpose(1, 0, 2).reshape(C, C)).astype(bf)
    wo3t = np.ascontiguousarray(
        wo3.T.reshape(H, D, C).transpose(1, 0, 2).reshape(C, C)).astype(bf)

    in_maps = []
    for i in range(NCORES):
        sl = slice(i * nloc, (i + 1) * nloc)
        # [nloc, K, C] -> [C, chunk, j, n]  (j-major within chunk)
        ei = e_m[sl].reshape(nchunks, CN, Kn, Cin)
        etc = np.ascontiguousarray(
            ei.transpose(3, 0, 2, 1).reshape(Cin, nloc * Kn)).astype(bf)
        xtc = np.ascontiguousarray(xf[sl].T).astype(bf)
        mc = np.ascontiguousarray(
            np.broadcast_to(Kn - cnt[sl], (C, nloc)).astype(bf))
        in_maps.append({
            "et": etc, "xt": xtc, "wq": wq, "wk": wk, "wv": wv,
            "hr": hrep, "idn": ident, "wos": wost, "wo3": wo3t, "mc": mc,
        })
    return in_maps, nloc


def assemble_output(results, B, N):
    BN = B * N
    nloc = BN // NCORES
    outf = np.empty((BN, C), np.float32)
    for i, r in enumerate(results):
        outf[i * nloc:(i + 1) * nloc] = r["out"].T
    return outf.reshape(B, N, C)


def kernel(h_X, h_E, mask_attn, W_Q, W_K, W_V, W_O):
    in_maps, nloc = prep_inputs(h_X, h_E, mask_attn, W_Q, W_K, W_V, W_O)
    nc = build_nc(nloc)
    res = run_bass_kernel_spmd(nc, in_maps, core_ids=list(range(NCORES)))
    B, N = np.asarray(h_X).shape[:2]
    return assemble_output(res.results, B, N)
